# revision 39
# baseline (speedup 1.0000x reference)
"""Distributed multi-head attention kernel for one TRN2 chip (8 NeuronCores).

Problem: x[2, 4096, 512] -> qkv proj (8 heads, dim 64) -> softmax attention
         -> out proj [2, 4096, 512].

Sharding (hardcoded): core c in 0..7 handles batch b = c // 4 and head pair
hp = c % 4 (heads 2*hp, 2*hp+1). Tensor-parallel: W_qkv column-sharded,
W_out row-sharded; each core emits a partial [4096, 512] output, the host
sums the 4 partials per batch and adds the bias.

Per-core kernel (all matmuls bf16, fp32 PSUM accumulation):

  x^T   arrives pre-transposed and pre-cast to bf16 from the host (one
        plain DMA per 512-column chunk straight into SBUF)
  KT,QT [d=128(2 heads), m] projections computed directly transposed
  V     [j, e] natural per j-tile, with a fused ones column per head
  S^T   [j-tile 128, m-block 1024] = KT_j^T @ QT  (K=64 per head)
  P^T   = exp(S^T * scale): 2/3 of tiles via one wide ACT instruction
        (PSUM -> bf16), 1/3 via a single DVE op computing the Schraudolph
        approximation int16(S*a+b) whose bits read as bf16 ~= exp — this
        splits the exp bottleneck (33.5M exps/core, ~266us on ACT alone)
        across two engines
  A     [m-tile, 64+1] = sum_j P^T_j^T @ [V_j | 1]  (K=128, full PE rate;
         the ones column yields the softmax denominator Z for free;
         the S matmul itself is PE output-rate-bound, so PE ~197us is the
         structural floor of this dataflow)
  then A/Z (DVE), transpose A (PE), @ W_out rows (PE), partial DMA out.

Phases are emitted interleaved (projection chunk n together with attention
j-tiles of the first block; the PV matmuls pipelined a few steps behind
their exps) and every persistent tensor is chunked into per-512-column
tiles: Tile tracks dependencies per tile and the engines are in-order, so
coarse tiles or phase-ordered emission would stall the exp stream for the
whole projection prologue. PSUM budget (8 banks) drives the structure:
S 2x2 + third S slot after the projections release their banks + 2
attention accumulators + projection scratch.
"""

import os

import numpy as np

L = 4096  # sequence length
D = 512  # model dim
HD = 64  # head dim
CB = 4  # contraction blocks for D (D / 128)
MT = L // 128  # 32 m-tiles (also j-tiles)
NCH = L // 512  # 8 column chunks
MBW = 1024  # m-block width for the attention streaming loop
NMB = L // MBW  # 4
NH = 2  # heads per core

_CACHE = {}

PV_DEPTH = int(os.environ.get("KERNEL_PV_DEPTH", "4"))
PT_BUFS = int(os.environ.get("KERNEL_PT_BUFS", "8"))
# DVE-offload ratio "N:K" = offload K of every N exp tiles (evenly spread)
_r = os.environ.get("KERNEL_EXP_DVE_RATIO", "3:1").split(":")
EXP_DVE_N, EXP_DVE_K = int(_r[0]), int(_r[1])


def _build(repeat=1):
    import concourse.tile as tile
    from concourse import bacc, mybir
    from concourse.masks import make_identity

    FP = mybir.dt.float32
    BF = mybir.dt.bfloat16

    nc = bacc.Bacc(
        "TRN2",
        target_bir_lowering=False,
        debug=False,
        enable_asserts=False,
        num_devices=8,
    )
    x = nc.dram_tensor("x", [D, L], BF, kind="ExternalInput").ap()
    wq = nc.dram_tensor("wq", [D, 128], BF, kind="ExternalInput").ap()
    wk = nc.dram_tensor("wk", [D, 128], BF, kind="ExternalInput").ap()
    wv = nc.dram_tensor("wv", [D, 128], BF, kind="ExternalInput").ap()
    wo = nc.dram_tensor("wo", [128, D], BF, kind="ExternalInput").ap()
    out = nc.dram_tensor("out", [L, D], FP, kind="ExternalOutput").ap()

    with tile.TileContext(nc) as tc:
        for _rep in range(repeat):
            _emit_body(nc, tc, mybir, make_identity, FP, BF, x, wq, wk, wv, wo, out)
    nc.compile()
    return nc


def _emit_body(nc, tc, mybir, make_identity, FP, BF, x, wq, wk, wv, wo, out):
    import math

    Exp = mybir.ActivationFunctionType.Exp
    I16 = mybir.dt.int16
    SCALE = float(HD**-0.5)
    # Schraudolph-style exp for the DVE-offloaded tiles: with y = round-ish
    # (x*scale*128/ln2 + (127*128 - 7)) written as int16, the bits of y read
    # as bf16 give ~exp(x*scale) (1.8% rms). One DVE op per tile; offloading
    # EXP_DVE_FRAC of tiles to DVE rebalances the ACT bottleneck.
    A_DVE = float(SCALE * 128.0 / math.log(2.0))
    B_DVE = float(127.0 * 128.0 - 7.0)

    with tc.tile_pool(name="singles", bufs=1) as singles:
        ident_b = singles.tile([128, 128], BF, tag="ident_b")
        make_identity(nc, ident_b)

        # chunked persistent activations (fine-grained scheduling deps)
        kt = [singles.tile([128, 512], BF, tag=f"kt{n}", name=f"kt{n}")
              for n in range(NCH)]
        qt = [singles.tile([128, 512], BF, tag=f"qt{n}", name=f"qt{n}")
              for n in range(NCH)]
        von = [singles.tile([128, 4, 130], BF, tag=f"von{n}", name=f"von{n}")
               for n in range(NCH)]
        anat = [singles.tile([128, 128], BF, tag=f"anat{m}", name=f"anat{m}")
                for m in range(MT)]

        # weights arrive pre-cast to bf16 from the host
        wq_b = singles.tile([128, CB, 128], BF, tag="wq_b")
        wk_b = singles.tile([128, CB, 128], BF, tag="wk_b")
        wv_b = singles.tile([128, CB, 128], BF, tag="wv_b")
        wo_b = singles.tile([128, D], BF, tag="wo_b")

        def emit_weight_loads_kq():
            nc.sync.dma_start(wk_b, wk.rearrange("(c p) d -> p c d", p=128))
            nc.sync.dma_start(wq_b, wq.rearrange("(c p) d -> p c d", p=128))

        def emit_weight_loads_vo():
            nc.sync.dma_start(wv_b, wv.rearrange("(c p) d -> p c d", p=128))
            nc.sync.dma_start(wo_b, wo[:, :])

        from contextlib import ExitStack

        pps_stack = ExitStack()
        with (
            tc.tile_pool(name="pxt", bufs=1) as p_xt,
            tc.tile_pool(name="p2s", bufs=2, space="PSUM") as p2s,
            tc.tile_pool(name="p2a", bufs=1, space="PSUM") as p2a,
            tc.tile_pool(name="p2pt", bufs=PT_BUFS) as p2pt,
            tc.tile_pool(name="p2n", bufs=3) as p2n,
            tc.tile_pool(name="ph3st", bufs=3) as p3st,
        ):
            p_ps = pps_stack.enter_context(
                tc.tile_pool(name="pps", bufs=1, space="PSUM")
            )
            xt = [p_xt.tile([128, CB, 512], BF, tag=f"xt{n}", name=f"xt{n}")
                  for n in range(NCH)]

            def emit_xchunk(n):
                # x arrives pre-transposed (and pre-cast) from the host: one
                # 512KB DMA straight into the x^T chunk tile.
                nc.sync.dma_start(
                    xt[n],
                    x[:, 512 * n : 512 * n + 512].rearrange(
                        "(c p) m -> p c m", p=128
                    ),
                )

            def emit_kqt(n):
                kp = p_ps.tile([128, 512], FP, tag="qkv")
                for c in range(CB):
                    nc.tensor.matmul(kp, lhsT=wk_b[:, c, :], rhs=xt[n][:, c, :],
                                     start=(c == 0), stop=(c == CB - 1))
                nc.vector.tensor_copy(kt[n], kp)
                qp = p_ps.tile([128, 512], FP, tag="qkv")
                for c in range(CB):
                    nc.tensor.matmul(qp, lhsT=wq_b[:, c, :], rhs=xt[n][:, c, :],
                                     start=(c == 0), stop=(c == CB - 1))
                nc.vector.tensor_copy(qt[n], qp)

            def emit_v(n):
                # V j-tiles 4n..4n+3, packed four per PSUM bank (one group)
                vp = p_ps.tile([128, 512], FP, tag="v")
                for jj in range(4):
                    for c in range(CB):
                        nc.tensor.matmul(
                            vp[:, 128 * jj : 128 * jj + 128],
                            lhsT=xt[n][:, c, 128 * jj : 128 * jj + 128],
                            rhs=wv_b[:, c, :],
                            start=(jj == 0 and c == 0),
                            stop=(jj == 3 and c == CB - 1),
                        )
                nc.gpsimd.memset(von[n][:, :, 64:65], 1.0)
                nc.gpsimd.memset(von[n][:, :, 129:130], 1.0)
                vp3 = vp.rearrange("p (a c) -> p a c", c=128)
                nc.vector.tensor_copy(von[n][:, :, 0:64], vp3[:, :, 0:64])
                nc.vector.tensor_copy(von[n][:, :, 65:129], vp3[:, :, 64:128])

            # ---- attention emitters (PV delayed one step behind S/exp) ----
            attn_tiles = {}
            pending = []

            def attn_slice(attnA, attnB, mt):
                if mt < 7:
                    return attnA[:, 65 * mt : 65 * mt + 65]
                return attnB

            exp_counter = [0]
            sp_pools = [None]

            def emit_s_exp(h, mb, j):
                if j == 0:
                    attn_tiles[(h, mb)] = (
                        p2a.tile([128, 7 * 65], FP, tag="attnA", name="attnA"),
                        p2a.tile([128, 65], FP, tag="attnB", name="attnB"),
                    )
                if sp_pools[0] is not None and exp_counter[0] % 3 == 2:
                    sp = sp_pools[0].tile([128, MBW], FP, tag="slate", name="sp")
                else:
                    sp = p2s.tile([128, MBW], FP, tag="s", name="sp")
                for half in range(MBW // 512):
                    nc.tensor.matmul(
                        sp[:, 512 * half : 512 * half + 512],
                        lhsT=kt[j // 4][
                            64 * h : 64 * h + 64,
                            128 * (j % 4) : 128 * (j % 4) + 128,
                        ],
                        rhs=qt[2 * mb + half][64 * h : 64 * h + 64, :],
                        start=True,
                        stop=True,
                    )
                exp_counter[0] += 1
                if EXP_DVE_K and (exp_counter[0] * EXP_DVE_K) % EXP_DVE_N < EXP_DVE_K:
                    # DVE path: int16(S*a + b) bits reinterpreted as bf16
                    pti = p2pt.tile([128, MBW], I16, tag="pt", name="pti")
                    nc.vector.tensor_scalar(
                        pti, sp, A_DVE, B_DVE,
                        mybir.AluOpType.mult, mybir.AluOpType.add,
                    )
                    pt = pti.bitcast(BF)
                else:
                    pt = p2pt.tile([128, MBW], BF, tag="pt", name="pt")
                    nc.scalar.activation(pt, sp, Exp, scale=SCALE)
                pending.append((h, mb, j, pt))

            def emit_pv(h, mb, j, pt):
                attnA, attnB = attn_tiles[(h, mb)]
                for mt in range(MBW // 128):
                    # one psum accumulation group per 2KB bank: attnA
                    # (mt 0..6) starts at (j0, mt0) and stops at (j31, mt6);
                    # attnB (mt 7) is its own bank.
                    if mt < 7:
                        mm_start = j == 0 and mt == 0
                        mm_stop = j == MT - 1 and mt == 6
                    else:
                        mm_start = j == 0
                        mm_stop = j == MT - 1
                    nc.tensor.matmul(
                        attn_slice(attnA, attnB, mt),
                        lhsT=pt[:, 128 * mt : 128 * mt + 128],
                        rhs=von[j // 4][:, j % 4, 65 * h : 65 * h + 65],
                        start=mm_start,
                        stop=mm_stop,
                    )
                if j == MT - 1:
                    # normalize by Z into the natural-layout bf16 A tiles
                    zr8 = p2n.tile([128, 8], FP, tag="zr", name="zr8")
                    za = attnA.rearrange("p (s c) -> p s c", c=65)
                    nc.vector.reciprocal(zr8[:, 0:7], za[:, :, 64])
                    nc.vector.reciprocal(zr8[:, 7:8], attnB[:, 64:65])
                    for mt in range(MBW // 128):
                        asl = attn_slice(attnA, attnB, mt)
                        gm = (MBW // 128) * mb + mt
                        nc.vector.tensor_scalar_mul(
                            anat[gm][:, 64 * h : 64 * h + 64],
                            asl[:, 0:64],
                            zr8[:, mt : mt + 1],
                        )
                    del attn_tiles[(h, mb)]

            def drain_pending(keep):
                while len(pending) > keep:
                    emit_pv(*pending.pop(0))

            # ---- interleaved emission ------------------------------------
            emit_xchunk(0)
            emit_xchunk(1)
            emit_weight_loads_kq()
            emit_xchunk(2)
            emit_weight_loads_vo()
            # warm up the PE clock (HAM releases the 1.2GHz throttle after
            # ~3.4us of activity) with junk transposes while DMAs land
            warm = p_ps.tile([128, 512], FP, tag="v", name="warm")
            wjunk = p2s.tile([128, MBW], BF, tag="s", name="wjunk")
            for w in range(20):
                nc.tensor.transpose(
                    wjunk[:, 128 * (w % 4) : 128 * (w % 4) + 128], ident_b, ident_b
                )
            emit_kqt(0)
            emit_kqt(1)
            emit_v(0)
            emit_v(1)
            for jj in range(4):
                emit_s_exp(0, 0, jj)
                drain_pending(1)
            for n in range(2, NCH):
                if n + 1 < NCH:
                    emit_xchunk(n + 1)
                emit_kqt(n)
                emit_v(n)
                for jj in range(4):
                    emit_s_exp(0, 0, 4 * (n - 1) + jj)
                    drain_pending(PV_DEPTH)
            # projections done: release their PSUM banks so the attention
            # stream gets a third S slot
            pps_stack.close()
            with (
                tc.tile_pool(name="p2slate", bufs=1, space="PSUM") as p2sl,
            ):
                sp_pools[0] = p2sl
                for j in range(4 * (NCH - 1), MT):
                    emit_s_exp(0, 0, j)
                    drain_pending(PV_DEPTH)
                blocks = [(1, 0)]
                for mb in range(1, NMB):
                    blocks += [(0, mb), (1, mb)]
                for h, mb in blocks:
                    for j in range(MT):
                        emit_s_exp(h, mb, j)
                        drain_pending(PV_DEPTH)
                drain_pending(0)

        # phase 3 entirely in the tail: attention PSUM pools closed, deep
        # pools, copies split across ACT and DVE
        with (
            tc.tile_pool(name="ph3at2", bufs=3, space="PSUM") as p3at2,
            tc.tile_pool(name="ph3ps2", bufs=3, space="PSUM") as p3ps2,
            tc.tile_pool(name="ph3st2", bufs=4) as p3st2,
        ):
            for mt in range(MT):
                atp = p3at2.tile([64, 256], BF, tag="atp2", name="atp2")
                nc.tensor.transpose(atp[:, 0:128], anat[mt][:, 0:64], ident_b)
                nc.tensor.transpose(atp[:, 128:256], anat[mt][:, 64:128], ident_b)
                att = p3st2.tile([128, 128], BF, tag="att2", name="att2")
                nc.scalar.copy(att[0:64, :], atp[:, 0:128])
                nc.vector.tensor_copy(att[64:128, :], atp[:, 128:256])
                op = p3ps2.tile([128, D], FP, tag="op2", name="op2")
                nc.tensor.matmul(op, lhsT=att, rhs=wo_b, start=True, stop=True)
                ost = p3st2.tile([128, D], FP, tag="ost2", name="ost2")
                if mt % 2 == 0:
                    nc.scalar.copy(ost, op)
                else:
                    nc.vector.tensor_copy(ost, op)
                nc.sync.dma_start(out[128 * mt : 128 * mt + 128, :], ost)


def _get_nc(repeat=1):
    key = ("nc", repeat)
    if key not in _CACHE:
        _CACHE[key] = _build(repeat)
    return _CACHE[key]


def kernel(x, W_qkv, W_out, b_out):
    import ml_dtypes

    from concourse.bass_utils import run_bass_kernel_spmd

    BF = ml_dtypes.bfloat16
    x = np.asarray(x, dtype=np.float32).astype(BF)
    W_qkv = np.asarray(W_qkv, dtype=np.float32).astype(BF)
    W_out = np.asarray(W_out, dtype=np.float32).astype(BF)
    b_out = np.asarray(b_out, dtype=np.float32)

    nc = _get_nc()

    in_maps = []
    for core in range(8):
        b = core // 4
        hp = core % 4
        cols = slice(128 * hp, 128 * hp + 128)
        in_maps.append(
            {
                "x": np.ascontiguousarray(x[b].T),
                "wq": np.ascontiguousarray(W_qkv[:, 0:512][:, cols]),
                "wk": np.ascontiguousarray(W_qkv[:, 512:1024][:, cols]),
                "wv": np.ascontiguousarray(W_qkv[:, 1024:1536][:, cols]),
                "wo": np.ascontiguousarray(W_out[cols, :]),
            }
        )

    trace = bool(int(os.environ.get("BASS_KERNEL_TRACE", "0")))
    res = run_bass_kernel_spmd(nc, in_maps, core_ids=list(range(8)), trace=trace)
    _CACHE["last_result"] = res

    outs = [np.asarray(r["out"], dtype=np.float32) for r in res.results]
    full = np.empty((2, L, D), dtype=np.float32)
    for b in range(2):
        acc = outs[4 * b].copy()
        for hp in range(1, 4):
            acc += outs[4 * b + hp]
        full[b] = acc + b_out
    return full


# revision 40
# speedup vs baseline: 1.0492x; 1.0492x over previous
"""Distributed multi-head attention kernel for one TRN2 chip (8 NeuronCores).

Problem: x[2, 4096, 512] -> qkv proj (8 heads, dim 64) -> softmax attention
         -> out proj [2, 4096, 512].

Sharding (hardcoded): core c in 0..7 handles batch b = c // 4 and head pair
hp = c % 4 (heads 2*hp, 2*hp+1). Tensor-parallel: W_qkv column-sharded,
W_out row-sharded; each core emits a partial [4096, 512] output, the host
sums the 4 partials per batch and adds the bias.

Per-core kernel (all matmuls bf16, fp32 PSUM accumulation):

  x^T   arrives pre-transposed and pre-cast to bf16 from the host (one
        plain DMA per 512-column chunk straight into SBUF)
  KT,QT [d=128(2 heads), m] projections computed directly transposed
  V     [j, e] natural per j-tile, with a fused ones column per head
  S^T   [j-tile 128, m-block 1024] = KT_j^T @ QT  (K=64 per head)
  P^T   = exp(S^T * scale): 2/3 of tiles via one wide ACT instruction
        (PSUM -> bf16), 1/3 via a single DVE op computing the Schraudolph
        approximation int16(S*a+b) whose bits read as bf16 ~= exp — this
        splits the exp bottleneck (33.5M exps/core, ~266us on ACT alone)
        across two engines
  A     [m-tile, 64+1] = sum_j P^T_j^T @ [V_j | 1]  (K=128, full PE rate;
         the ones column yields the softmax denominator Z for free;
         the S matmul itself is PE output-rate-bound, so PE ~197us is the
         structural floor of this dataflow)
  then A/Z (DVE), transpose A (PE), @ W_out rows (PE), partial DMA out.

Phases are emitted interleaved (projection chunk n together with attention
j-tiles of the first block; the PV matmuls pipelined a few steps behind
their exps) and every persistent tensor is chunked into per-512-column
tiles: Tile tracks dependencies per tile and the engines are in-order, so
coarse tiles or phase-ordered emission would stall the exp stream for the
whole projection prologue. PSUM budget (8 banks) drives the structure:
S 2x2 + third S slot after the projections release their banks + 2
attention accumulators + projection scratch.
"""

import os

import numpy as np

L = 4096  # sequence length
D = 512  # model dim
HD = 64  # head dim
CB = 4  # contraction blocks for D (D / 128)
MT = L // 128  # 32 m-tiles (also j-tiles)
NCH = L // 512  # 8 column chunks
MBW = 1024  # m-block width for the attention streaming loop
NMB = L // MBW  # 4
NH = 2  # heads per core

_CACHE = {}

PV_DEPTH = int(os.environ.get("KERNEL_PV_DEPTH", "4"))
PT_BUFS = int(os.environ.get("KERNEL_PT_BUFS", "8"))
# DVE-offload ratio "N:K" = offload K of every N exp tiles (evenly spread)
_r = os.environ.get("KERNEL_EXP_DVE_RATIO", "3:1").split(":")
EXP_DVE_N, EXP_DVE_K = int(_r[0]), int(_r[1])


def _build(repeat=1):
    import concourse.tile as tile
    from concourse import bacc, mybir
    from concourse.masks import make_identity

    FP = mybir.dt.float32
    BF = mybir.dt.bfloat16

    nc = bacc.Bacc(
        "TRN2",
        target_bir_lowering=False,
        debug=False,
        enable_asserts=False,
        num_devices=8,
    )
    x = nc.dram_tensor("x", [D, L], BF, kind="ExternalInput").ap()
    wq = nc.dram_tensor("wq", [D, 128], BF, kind="ExternalInput").ap()
    wk = nc.dram_tensor("wk", [D, 128], BF, kind="ExternalInput").ap()
    wv = nc.dram_tensor("wv", [D, 128], BF, kind="ExternalInput").ap()
    wo = nc.dram_tensor("wo", [128, D], BF, kind="ExternalInput").ap()
    out = nc.dram_tensor("out", [L, D], FP, kind="ExternalOutput").ap()

    with tile.TileContext(nc) as tc:
        for _rep in range(repeat):
            _emit_body(nc, tc, mybir, make_identity, FP, BF, x, wq, wk, wv, wo, out)
    nc.compile()
    return nc


def _emit_body(nc, tc, mybir, make_identity, FP, BF, x, wq, wk, wv, wo, out):
    import math

    Exp = mybir.ActivationFunctionType.Exp
    I16 = mybir.dt.int16
    SCALE = float(HD**-0.5)
    # Schraudolph-style exp for the DVE-offloaded tiles: with y = round-ish
    # (x*scale*128/ln2 + (127*128 - 7)) written as int16, the bits of y read
    # as bf16 give ~exp(x*scale) (1.8% rms). One DVE op per tile; offloading
    # EXP_DVE_FRAC of tiles to DVE rebalances the ACT bottleneck.
    A_DVE = float(SCALE * 128.0 / math.log(2.0))
    B_DVE = float(127.0 * 128.0 - 7.0)

    with tc.tile_pool(name="singles", bufs=1) as singles:
        ident_b = singles.tile([128, 128], BF, tag="ident_b")
        make_identity(nc, ident_b)

        # chunked persistent activations (fine-grained scheduling deps)
        kt = [singles.tile([128, 512], BF, tag=f"kt{n}", name=f"kt{n}")
              for n in range(NCH)]
        qt = [singles.tile([128, 512], BF, tag=f"qt{n}", name=f"qt{n}")
              for n in range(NCH)]
        von = [singles.tile([128, 4, 130], BF, tag=f"von{n}", name=f"von{n}")
               for n in range(NCH)]
        anat = [singles.tile([128, 128], BF, tag=f"anat{m}", name=f"anat{m}")
                for m in range(MT)]

        # weights arrive pre-cast to bf16 from the host
        wq_b = singles.tile([128, CB, 128], BF, tag="wq_b")
        wk_b = singles.tile([128, CB, 128], BF, tag="wk_b")
        wv_b = singles.tile([128, CB, 128], BF, tag="wv_b")
        wo_b = singles.tile([128, D], BF, tag="wo_b")

        def emit_weight_loads_kq():
            nc.sync.dma_start(wk_b, wk.rearrange("(c p) d -> p c d", p=128))
            nc.sync.dma_start(wq_b, wq.rearrange("(c p) d -> p c d", p=128))

        def emit_weight_loads_vo():
            nc.sync.dma_start(wv_b, wv.rearrange("(c p) d -> p c d", p=128))
            nc.sync.dma_start(wo_b, wo[:, :])

        from contextlib import ExitStack

        pps_stack = ExitStack()
        with (
            tc.tile_pool(name="pxt", bufs=1) as p_xt,
            tc.tile_pool(name="p2s", bufs=2, space="PSUM") as p2s,
            tc.tile_pool(name="p2a", bufs=1, space="PSUM") as p2a,
            tc.tile_pool(name="p2pt", bufs=PT_BUFS) as p2pt,
            tc.tile_pool(name="p2n", bufs=3) as p2n,
            tc.tile_pool(name="ph3st", bufs=3) as p3st,
        ):
            p_ps = pps_stack.enter_context(
                tc.tile_pool(name="pps", bufs=1, space="PSUM")
            )
            xt = [p_xt.tile([128, CB, 512], BF, tag=f"xt{n}", name=f"xt{n}")
                  for n in range(NCH)]

            def emit_xchunk(n):
                # x arrives pre-transposed (and pre-cast) from the host: one
                # 512KB DMA straight into the x^T chunk tile.
                nc.sync.dma_start(
                    xt[n],
                    x[:, 512 * n : 512 * n + 512].rearrange(
                        "(c p) m -> p c m", p=128
                    ),
                )

            def emit_kqt(n):
                # chunks 0-1 copy on the (pre-exp idle) ACT engine: shortens
                # the prologue critical chain and keeps DVE free for its
                # first offloaded exps
                copy = nc.scalar.copy if n < 2 else nc.vector.tensor_copy
                kp = p_ps.tile([128, 512], FP, tag="qkv")
                for c in range(CB):
                    nc.tensor.matmul(kp, lhsT=wk_b[:, c, :], rhs=xt[n][:, c, :],
                                     start=(c == 0), stop=(c == CB - 1))
                copy(kt[n], kp)
                qp = p_ps.tile([128, 512], FP, tag="qkv")
                for c in range(CB):
                    nc.tensor.matmul(qp, lhsT=wq_b[:, c, :], rhs=xt[n][:, c, :],
                                     start=(c == 0), stop=(c == CB - 1))
                copy(qt[n], qp)

            def emit_v(n):
                # V j-tiles 4n..4n+3, packed four per PSUM bank (one group)
                vp = p_ps.tile([128, 512], FP, tag="v")
                for jj in range(4):
                    for c in range(CB):
                        nc.tensor.matmul(
                            vp[:, 128 * jj : 128 * jj + 128],
                            lhsT=xt[n][:, c, 128 * jj : 128 * jj + 128],
                            rhs=wv_b[:, c, :],
                            start=(jj == 0 and c == 0),
                            stop=(jj == 3 and c == CB - 1),
                        )
                nc.gpsimd.memset(von[n][:, :, 64:65], 1.0)
                nc.gpsimd.memset(von[n][:, :, 129:130], 1.0)
                vp3 = vp.rearrange("p (a c) -> p a c", c=128)
                nc.vector.tensor_copy(von[n][:, :, 0:64], vp3[:, :, 0:64])
                nc.vector.tensor_copy(von[n][:, :, 65:129], vp3[:, :, 64:128])

            # ---- attention emitters (PV delayed one step behind S/exp) ----
            attn_tiles = {}
            pending = []

            def attn_slice(attnA, attnB, mt):
                if mt < 7:
                    return attnA[:, 65 * mt : 65 * mt + 65]
                return attnB

            exp_counter = [0]
            sp_pools = [None]

            def emit_s_exp(h, mb, j):
                if j == 0:
                    attn_tiles[(h, mb)] = (
                        p2a.tile([128, 7 * 65], FP, tag="attnA", name="attnA"),
                        p2a.tile([128, 65], FP, tag="attnB", name="attnB"),
                    )
                if sp_pools[0] is not None and exp_counter[0] % 3 == 2:
                    sp = sp_pools[0].tile([128, MBW], FP, tag="slate", name="sp")
                else:
                    sp = p2s.tile([128, MBW], FP, tag="s", name="sp")
                for half in range(MBW // 512):
                    nc.tensor.matmul(
                        sp[:, 512 * half : 512 * half + 512],
                        lhsT=kt[j // 4][
                            64 * h : 64 * h + 64,
                            128 * (j % 4) : 128 * (j % 4) + 128,
                        ],
                        rhs=qt[2 * mb + half][64 * h : 64 * h + 64, :],
                        start=True,
                        stop=True,
                    )
                exp_counter[0] += 1
                if EXP_DVE_K and (exp_counter[0] * EXP_DVE_K) % EXP_DVE_N < EXP_DVE_K:
                    # DVE path: int16(S*a + b) bits reinterpreted as bf16
                    pti = p2pt.tile([128, MBW], I16, tag="pt", name="pti")
                    nc.vector.tensor_scalar(
                        pti, sp, A_DVE, B_DVE,
                        mybir.AluOpType.mult, mybir.AluOpType.add,
                    )
                    pt = pti.bitcast(BF)
                else:
                    pt = p2pt.tile([128, MBW], BF, tag="pt", name="pt")
                    nc.scalar.activation(pt, sp, Exp, scale=SCALE)
                pending.append((h, mb, j, pt))

            def emit_pv(h, mb, j, pt):
                attnA, attnB = attn_tiles[(h, mb)]
                for mt in range(MBW // 128):
                    # one psum accumulation group per 2KB bank: attnA
                    # (mt 0..6) starts at (j0, mt0) and stops at (j31, mt6);
                    # attnB (mt 7) is its own bank.
                    if mt < 7:
                        mm_start = j == 0 and mt == 0
                        mm_stop = j == MT - 1 and mt == 6
                    else:
                        mm_start = j == 0
                        mm_stop = j == MT - 1
                    nc.tensor.matmul(
                        attn_slice(attnA, attnB, mt),
                        lhsT=pt[:, 128 * mt : 128 * mt + 128],
                        rhs=von[j // 4][:, j % 4, 65 * h : 65 * h + 65],
                        start=mm_start,
                        stop=mm_stop,
                    )
                if j == MT - 1:
                    # normalize by Z into the natural-layout bf16 A tiles
                    zr8 = p2n.tile([128, 8], FP, tag="zr", name="zr8")
                    za = attnA.rearrange("p (s c) -> p s c", c=65)
                    nc.vector.reciprocal(zr8[:, 0:7], za[:, :, 64])
                    nc.vector.reciprocal(zr8[:, 7:8], attnB[:, 64:65])
                    for mt in range(MBW // 128):
                        asl = attn_slice(attnA, attnB, mt)
                        gm = (MBW // 128) * mb + mt
                        nc.vector.tensor_scalar_mul(
                            anat[gm][:, 64 * h : 64 * h + 64],
                            asl[:, 0:64],
                            zr8[:, mt : mt + 1],
                        )
                    del attn_tiles[(h, mb)]

            def drain_pending(keep):
                while len(pending) > keep:
                    emit_pv(*pending.pop(0))

            # ---- interleaved emission ------------------------------------
            emit_xchunk(0)
            emit_xchunk(1)
            emit_weight_loads_kq()
            emit_xchunk(2)
            emit_weight_loads_vo()
            # warm up the PE clock (HAM releases the 1.2GHz throttle after
            # ~3.4us of activity) with junk transposes while DMAs land
            warm = p_ps.tile([128, 512], FP, tag="v", name="warm")
            wjunk = p2s.tile([128, MBW], BF, tag="s", name="wjunk")
            for w in range(20):
                nc.tensor.transpose(
                    wjunk[:, 128 * (w % 4) : 128 * (w % 4) + 128], ident_b, ident_b
                )
            emit_kqt(0)
            emit_kqt(1)
            emit_v(0)
            emit_v(1)
            for jj in range(4):
                emit_s_exp(0, 0, jj)
                drain_pending(1)
            for n in range(2, NCH):
                if n + 1 < NCH:
                    emit_xchunk(n + 1)
                emit_kqt(n)
                emit_v(n)
                for jj in range(4):
                    emit_s_exp(0, 0, 4 * (n - 1) + jj)
                    drain_pending(PV_DEPTH)
            # projections done: release their PSUM banks so the attention
            # stream gets a third S slot
            pps_stack.close()
            with (
                tc.tile_pool(name="p2slate", bufs=1, space="PSUM") as p2sl,
            ):
                sp_pools[0] = p2sl
                for j in range(4 * (NCH - 1), MT):
                    emit_s_exp(0, 0, j)
                    drain_pending(PV_DEPTH)
                blocks = [(1, 0)]
                for mb in range(1, NMB):
                    blocks += [(0, mb), (1, mb)]
                for h, mb in blocks:
                    for j in range(MT):
                        emit_s_exp(h, mb, j)
                        drain_pending(PV_DEPTH)
                drain_pending(0)

        # phase 3 entirely in the tail: attention PSUM pools closed, deep
        # pools, copies split across ACT and DVE
        with (
            tc.tile_pool(name="ph3at2", bufs=3, space="PSUM") as p3at2,
            tc.tile_pool(name="ph3ps2", bufs=3, space="PSUM") as p3ps2,
            tc.tile_pool(name="ph3st2", bufs=4) as p3st2,
        ):
            for mt in range(MT):
                atp = p3at2.tile([64, 256], BF, tag="atp2", name="atp2")
                nc.tensor.transpose(atp[:, 0:128], anat[mt][:, 0:64], ident_b)
                nc.tensor.transpose(atp[:, 128:256], anat[mt][:, 64:128], ident_b)
                att = p3st2.tile([128, 128], BF, tag="att2", name="att2")
                nc.scalar.copy(att[0:64, :], atp[:, 0:128])
                nc.vector.tensor_copy(att[64:128, :], atp[:, 128:256])
                op = p3ps2.tile([128, D], FP, tag="op2", name="op2")
                nc.tensor.matmul(op, lhsT=att, rhs=wo_b, start=True, stop=True)
                ost = p3st2.tile([128, D], FP, tag="ost2", name="ost2")
                if mt % 2 == 0:
                    nc.scalar.copy(ost, op)
                else:
                    nc.vector.tensor_copy(ost, op)
                nc.sync.dma_start(out[128 * mt : 128 * mt + 128, :], ost)


def _get_nc(repeat=1):
    key = ("nc", repeat)
    if key not in _CACHE:
        _CACHE[key] = _build(repeat)
    return _CACHE[key]


def kernel(x, W_qkv, W_out, b_out):
    import ml_dtypes

    from concourse.bass_utils import run_bass_kernel_spmd

    BF = ml_dtypes.bfloat16
    x = np.asarray(x, dtype=np.float32).astype(BF)
    W_qkv = np.asarray(W_qkv, dtype=np.float32).astype(BF)
    W_out = np.asarray(W_out, dtype=np.float32).astype(BF)
    b_out = np.asarray(b_out, dtype=np.float32)

    nc = _get_nc()

    in_maps = []
    for core in range(8):
        b = core // 4
        hp = core % 4
        cols = slice(128 * hp, 128 * hp + 128)
        in_maps.append(
            {
                "x": np.ascontiguousarray(x[b].T),
                "wq": np.ascontiguousarray(W_qkv[:, 0:512][:, cols]),
                "wk": np.ascontiguousarray(W_qkv[:, 512:1024][:, cols]),
                "wv": np.ascontiguousarray(W_qkv[:, 1024:1536][:, cols]),
                "wo": np.ascontiguousarray(W_out[cols, :]),
            }
        )

    trace = bool(int(os.environ.get("BASS_KERNEL_TRACE", "0")))
    res = run_bass_kernel_spmd(nc, in_maps, core_ids=list(range(8)), trace=trace)
    _CACHE["last_result"] = res

    outs = [np.asarray(r["out"], dtype=np.float32) for r in res.results]
    full = np.empty((2, L, D), dtype=np.float32)
    for b in range(2):
        acc = outs[4 * b].copy()
        for hp in range(1, 4):
            acc += outs[4 * b + hp]
        full[b] = acc + b_out
    return full
